# revision 5
# baseline (speedup 1.0000x reference)
"""Trainium2 Bass kernel for nn_CombinatorialClassifier.

Computation (reference):
    logits = einsum('bf,pqf->bpq', x, W) + b        # [B,P,Q]
    logp   = log_softmax(logits, axis=2)            # [B,P,Q]
    out    = take_along_axis(logp, part_idx, 2)     # [B,P,C]

Shapes: B=256, P=64, Q=128, C=1000, F=2048.  Expert-parallel over P
(8 partitionings per core), full x on every core, no collectives.

Design (~1.7x faster than the v1 one-hot/orientation-A kernel):
  - main matmul in orientation B: psum_lin[b, (p,q)] += x_k.T @ W_k
    with x stationary and fp8e4 DoubleRow (2 contraction rows per
    partition: half the HBM bytes, 2x PE rate; rel err 6.8e-3 vs the
    2e-2 gate).  W is scaled by 32 on the host; the 1/32 descale rides
    the ACT/DVE copies.  Mains stream k-major behind the input DMAs.
  - softmax: wide ACT exps + DVE tensor_reduce give sumexp in the
    [b, 1] orientation for free (no PE sumexp, no lse transposes); lse
    is folded per-partition into the og drain copies, so the gather
    operates on raw rescaled logits.
  - gather is a one-hot matmul; the one-hot is host-built fp8e4 (exact
    0/1, mixed bf16 x fp8 operands).  Rescaled logits are transposed
    to [q, b] via PE transposes (identity DMA'd once) into per-blk
    bf16 psum tiles, copied to SBUF by DVE.
  - the psum->SBUF og drain alternates per 500-col chunk between DVE
    (tensor_scalar subtract) and ACT (Identity + neg_lse bias) over 6
    rotating one-bank psum slots, so both engines drain concurrently
    and every gather matmul carries exactly ONE semaphore wait (this
    walrus build encodes at most one sync wait per instruction; the
    chunk-class/slot-parity assignment makes each slot WAR land on the
    same semaphore as the gather's data wait).
  - outputs are bf16 halves (host stitches + upcasts): c0 via SP-issued
    HWDGE DMAs (the input queue is provably drained, so the legalizer
    drops the queue-pred wait), c1 via Pool-issued SWDGE DMAs behind a
    Pool observer.
  - _install_drain_split post-processes the serialized BIR with a
    vector-clock pass that drops transitively-implied semaphore waits
    and splits the tail Drains, enforcing the single-wait encoding.
"""

import numpy as np

B, P, Q, C, F = 256, 64, 128, 1000, 2048
NCORES = 8
PL = P // NCORES          # partitionings per core
KT2 = 8                   # K tiles of 256 (128 partitions x DoubleRow 2)
XC = B                    # x columns in the xw stream
WC = PL * Q               # W columns in the xw stream
NBLK = B // 128           # b blocks
WSCALE = 32.0


def _build_nc():
    import concourse.bass as bass
    import concourse.tile as tile
    from concourse import mybir
    from contextlib import ExitStack

    DT = mybir.dt.float32
    BF = mybir.dt.bfloat16
    F8 = mybir.dt.float8e4
    ACT = mybir.ActivationFunctionType

    nc = bass.Bass()
    xw_d = nc.declare_dram_parameter("xw", [KT2, 128, 2, XC + WC], F8,
                                     isOutput=False)
    bo_d = nc.declare_dram_parameter("bo", [1, WC + 128], BF, isOutput=False)
    id_d = nc.declare_dram_parameter("ident", [128, 128], BF, isOutput=False)
    oh_d = nc.declare_dram_parameter("oh", [128, PL * C], F8, isOutput=False)
    out_d = nc.declare_dram_parameter("out", [B, PL, 500], BF, isOutput=True)
    out2_d = nc.declare_dram_parameter("out2", [B, PL, 500], BF,
                                       isOutput=True)

    with ExitStack() as ctx:
        tc = ctx.enter_context(tile.TileContext(nc))
        singles = ctx.enter_context(tc.tile_pool(name="singles", bufs=1))
        ps_t = ctx.enter_context(
            tc.tile_pool(name="ps_t", bufs=1, space=bass.MemorySpace.PSUM))
        lin_ctx = ExitStack()
        ps_lin = lin_ctx.enter_context(
            tc.tile_pool(name="ps_lin", bufs=1, space=bass.MemorySpace.PSUM))

        def fresh(shape, dtype, tag):
            return singles.tile(shape, dtype, tag=tag, name=tag)

        # ---- input DMAs, all on the SP HWDGE queue (ordered sems) ---
        bo_sb = fresh([1, WC + 128], BF, "bo")
        nc.sync.dma_start(out=bo_sb[:], in_=bo_d[:])
        id_sb = fresh([128, 128], BF, "ident")
        nc.sync.dma_start(out=id_sb[:], in_=id_d[:])
        xwk = []
        for k in range(KT2):
            t = fresh([128, 2, XC + WC], F8, f"xwk{k}")
            nc.sync.dma_start(out=t[:], in_=xw_d[k])
            xwk.append(t)
        oh_sb = fresh([128, PL * C], F8, "oh")
        nc.sync.dma_start(out=oh_sb[:], in_=oh_d[:])

        pst = {}
        for blk in range(NBLK):
            pst[blk] = ps_t.tile([128, PL, 128], BF, name=f"pst{blk}")

        # ---- PE: warmups, bias openers + DoubleRow mains -------------
        lin = {}
        for blk in range(NBLK):
            for h in range(2):
                t = ps_lin.tile([128, 4, 128], DT, name=f'lin{blk}_{h}')
                lin[(blk, h)] = t
        # dummy transpose consumes ident's DMA sem early
        nc.tensor.transpose(pst[0][:, 0, :], id_sb[:], id_sb[:])
        for blk in range(NBLK):
            for h in range(2):
                nc.tensor.matmul(
                    lin[(blk, h)][:, :, :],
                    bo_sb[0:1, WC:WC + 128],
                    bo_sb[0:1, h * 512:(h + 1) * 512],
                    start=True, stop=False)
        for blk in range(NBLK):
            for k in range(KT2):
                for h in range(2):
                    nc.tensor.matmul(
                        lin[(blk, h)][:, :, :],
                        xwk[k][:, :, blk * 128:(blk + 1) * 128],
                        xwk[k][:, :, XC + h * 512:XC + (h + 1) * 512],
                        start=False, stop=(k == KT2 - 1),
                        perf_mode=mybir.MatmulPerfMode.DoubleRow)

        # PE observer: absorb oh's DMA sem so gathers carry only their
        # drain-chain wait
        nc.tensor.ldweights(oh_sb[:, 0:1])

        # ---- softmax prologue per blk --------------------------------
        obs_junk = fresh([1, 8], DT, "obs_junk")
        warm_junk = fresh([1, 2], DT, "warm_junk")
        nc.scalar.activation(out=warm_junk[0:1, 0:1],
                             in_=warm_junk[0:1, 1:2],
                             func=ACT.Identity, scale=0.0, bias=0.0)

        linsc, lse, neg_lse, exps, sumexp = {}, {}, {}, {}, {}

        def mk_linsc(blk, eng):
            for h in range(2):
                t = fresh([128, 4, 128], BF, f"linsc{blk}_{h}")
                linsc[(blk, h)] = t
                if eng == 'act':
                    nc.scalar.activation(out=t[:, :, :],
                                         in_=lin[(blk, h)][:, :, :],
                                         func=ACT.Copy, scale=1.0 / WSCALE,
                                         bias=0.0)
                else:
                    nc.vector.tensor_scalar_mul(t[:, :, :],
                                                lin[(blk, h)][:, :, :],
                                                1.0 / WSCALE)

        def act_exp(blk):
            for h in range(2):
                e = fresh([128, 4, 128], BF, f"exp{blk}_{h}")
                exps[(blk, h)] = e
                nc.scalar.activation(out=e[:, :, :], in_=lin[(blk, h)][:, :, :],
                                     func=ACT.Exp, scale=1.0 / WSCALE)

        def dve_red(blk):
            sumexp[blk] = fresh([128, PL], DT, f"sumexp{blk}")
            for h in range(2):
                nc.vector.tensor_reduce(
                    out=sumexp[blk][:, h * 4:(h + 1) * 4],
                    in_=exps[(blk, h)][:, :, :],
                    axis=mybir.AxisListType.X, op=mybir.AluOpType.add)

        def act_post(blk):
            t = fresh([128, PL], DT, f"lse{blk}")
            lse[blk] = t
            nc.scalar.activation(out=t[:], in_=sumexp[blk][:], func=ACT.Ln)
            t2 = fresh([128, PL], DT, f"neglse{blk}")
            neg_lse[blk] = t2
            nc.scalar.activation(out=t2[:], in_=lse[blk][:],
                                 func=ACT.Identity, scale=-1.0)
            # ACT self-absorber for the neg_lse RAW
            aabs = fresh([1, 1], DT, f"aabs{blk}")
            nc.scalar.activation(out=aabs[:], in_=neg_lse[blk][0:1, 0:1],
                                 func=ACT.Copy, bias=0.0, scale=1.0)

        act_exp(0)
        act_exp(1)
        mk_linsc(0, 'act')
        mk_linsc(1, 'act')
        dve_red(0)
        dve_red(1)

        # ---- transposes + logpT copies (all copies on DVE) -----------
        logpT = {}
        for blk in range(NBLK):
            for p in range(PL):
                nc.tensor.transpose(pst[blk][:, p, :],
                                    linsc[(blk, p // 4)][:, p % 4, :],
                                    id_sb[:])
        for blk in range(NBLK):
            for p in range(PL):
                t = fresh([128, 128], BF, f"logpT{blk}_{p}")
                logpT[(blk, p)] = t
                nc.vector.tensor_copy(t[:], pst[blk][:, p, :])
        act_post(0)
        act_post(1)
        # DVE absorbers: pull ACT@neg_lse into DVE's clock before ogs
        dabs0 = fresh([128, 1], DT, "dabs0")
        nc.vector.tensor_copy(dabs0[:], neg_lse[0][:, 0:1])
        dabs1 = fresh([128, 1], DT, "dabs1")
        nc.vector.tensor_copy(dabs1[:], neg_lse[1][:, 0:1])

        # lin banks free; 6 one-bank chunk slots (pst banks stay live)
        lin_ctx.close()
        ps_out = ctx.enter_context(
            tc.tile_pool(name="ps_out", bufs=6, space=bass.MemorySpace.PSUM))

        # PE LDW observer: absorb ACT's lin-bank reads (exps + linscs)
        # so first-rotation gathers' bank-reuse WARs are covered
        nc.tensor.ldweights(linsc[(1, 1)][:, 0, 0:1])


        og_c0, og_c1 = {}, {}
        og_c0_last, og_c1_last = {}, {}

        def gather_p(blk, p):
            pair = p // 2
            if p % 2 == 0:
                og_c0[(blk, pair)] = fresh([128, 2, 500], BF,
                                           f"ogc0_{blk}_{pair}")
                og_c1[(blk, pair)] = fresh([128, 2, 500], BF,
                                           f"ogc1_{blk}_{pair}")
            for ci in range(2):
                po = ps_out.tile([128, 512], DT, name='po')
                nc.tensor.matmul(
                    po[:, 0:500], logpT[(blk, p)][:],
                    oh_sb[:, p * C + ci * 500:p * C + ci * 500 + 500],
                    start=True, stop=True)
                if ci == 0:
                    # c0 chunks drain on DVE (fused lse subtract)
                    og_c0_last[(blk, pair)] = nc.vector.tensor_scalar(
                        out=og_c0[(blk, pair)][:, p % 2, :],
                        in0=po[:, 0:500],
                        scalar1=lse[blk][:, p:p + 1], scalar2=None,
                        op0=mybir.AluOpType.subtract)
                else:
                    # c1 chunks drain on ACT
                    og_c1_last[(blk, pair)] = nc.scalar.activation(
                        out=og_c1[(blk, pair)][:, p % 2, :], in_=po[:, 0:500],
                        func=ACT.Identity, scale=1.0,
                        bias=neg_lse[blk][:, p:p + 1])
            if p % 2 == 1:
                bsl = slice(blk * 128, (blk + 1) * 128)
                # c0 out: SP-issued (input queue provably drained, so the
                # legalizer drops the queue-pred wait; single DVE sem)
                dma0 = nc.sync.dma_start(
                    out=out_d[bsl, pair * 2:pair * 2 + 2, :],
                    in_=og_c0[(blk, pair)][:])
                tile.add_dep_helper(dma0.ins, og_c0_last[(blk, pair)].ins,
                                    sync=False, reason="dma after og c0")
                # c1 out: Pool-issued SWDGE behind a Pool observer
                obs = nc.gpsimd.tensor_copy(
                    obs_junk[0:1, blk * 4 + pair:blk * 4 + pair + 1],
                    og_c1[(blk, pair)][0:1, 1, 499:500])
                dma1 = nc.gpsimd.dma_start(
                    out=out2_d[bsl, pair * 2:pair * 2 + 2, :],
                    in_=og_c1[(blk, pair)][:])
                tile.add_dep_helper(dma1.ins, obs.ins, sync=False,
                                    reason="dma after pool obs")

        # ---- per-p interleaved gathers: both drain chains run hot ----
        for p in range(PL):
            gather_p(0, p)
            gather_p(1, p)

    _install_drain_split(nc)
    return nc


def _install_drain_split(nc, chunk=1):
    """Legalize sync for this walrus build (at most ONE sync wait per
    instruction):

    1. Vector-clock pass: compute, for every instruction, the set of
       instructions provably COMPLETED before it dispatches — via its
       own sem waits (a wait S>=v proves every update contributing to
       values 1..v completed, and transitively everything those
       instructions' dispatch-clocks contain) plus same-engine dispatch
       order (an engine dispatches in program order, so anything done
       before a predecessor's dispatch is done before ours).  Any
       emitted wait already implied by the rest is dropped.
    2. Remaining multi-wait Drains are split into single-wait chains.
    """
    import copy
    import json

    orig = nc.to_json_bytes

    def patched():
        m = json.loads(orig())
        for fn in m["functions"]:
            insts = []
            for bb in fn["blocks"]:
                insts.extend(bb["instructions"])
            n = len(insts)
            # sem name -> list of (cum_value, idx) in completion order
            updates = {}
            cum = {}
            for i, inst in enumerate(insts):
                si = inst.get("sync_info") or {}
                for u in (si.get("on_update") or []):
                    s = u["ant_name"]
                    cum[s] = cum.get(s, 0) + u.get("update_value", 1)
                    updates.setdefault(s, []).append((cum[s], i))

            def targets(s, v):
                """instruction idxs whose updates are needed for sem s
                to reach v"""
                return [i for (c, i) in updates.get(s, []) if c <= v]

            eng_pred = {}
            last = {}
            for i, inst in enumerate(insts):
                e = inst.get("engine", "?")
                eng_pred[i] = last.get(e)
                last[e] = i

            done = [set() for _ in range(n)]
            for _ in range(64):
                changed = False
                for i, inst in enumerate(insts):
                    d = set()
                    if eng_pred[i] is not None:
                        p = eng_pred[i]
                        d |= done[p]
                    si = inst.get("sync_info") or {}
                    for w in (si.get("on_wait") or []):
                        for j in targets(w["ant_name"], w["wait_value"]):
                            d.add(j)
                            d |= done[j]
                    if d != done[i]:
                        done[i] = d
                        changed = True
                if not changed:
                    break

            # drop implied waits on multi-wait instructions
            for i, inst in enumerate(insts):
                si = inst.get("sync_info") or {}
                waits = si.get("on_wait") or []
                if len(waits) <= 1:
                    continue
                base = set()
                if eng_pred[i] is not None:
                    base |= done[eng_pred[i]]
                keep = list(waits)
                for w in list(keep):
                    others = set(base)
                    for w2 in keep:
                        if w2 is w:
                            continue
                        for j in targets(w2["ant_name"], w2["wait_value"]):
                            others.add(j)
                            others |= done[j]
                    if all(j in others
                           for j in targets(w["ant_name"], w["wait_value"])):
                        keep.remove(w)
                        if len(keep) <= 1:
                            break
                si["on_wait"] = keep

        # split any remaining multi-wait Drains
        for fn in m["functions"]:
            for bb in fn["blocks"]:
                out = []
                for inst in bb["instructions"]:
                    si = inst.get("sync_info")
                    if (si and si.get("on_wait")
                            and len(si["on_wait"]) > chunk):
                        if inst.get("opcode") != "Drain":
                            raise RuntimeError(
                                f"multi-wait survives legalization: "
                                f"{inst.get('opcode')} {inst.get('name')} "
                                f"{si['on_wait']}")
                        waits = si["on_wait"]
                        head, keep = waits[:-chunk], waits[-chunk:]
                        for j in range(0, len(head), chunk):
                            clone = copy.deepcopy(inst)
                            clone["name"] = f"{inst['name']}-ds{j}"
                            clone["sync_info"] = {
                                "on_wait": head[j:j + chunk],
                                "on_update": [],
                            }
                            out.append(clone)
                        si["on_wait"] = keep
                    out.append(inst)
                bb["instructions"] = out
        return json.dumps(m).encode()

    nc.to_json_bytes = patched


def _host_inputs(x, W, b, part_idx):
    import ml_dtypes
    f8 = ml_dtypes.float8_e4m3
    bf = ml_dtypes.bfloat16

    # x: [B, F] -> [KT2, 128, 2, B] with f = 256*k2 + 128*j + r
    xT = np.ascontiguousarray(
        x.T.reshape(KT2, 2, 128, B).transpose(0, 2, 1, 3)).astype(f8)
    ident = np.eye(128, dtype=np.float32).astype(bf)
    qarange = np.arange(Q, dtype=np.int64)

    in_maps = []
    for i in range(NCORES):
        sl = slice(i * PL, (i + 1) * PL)
        Wt = (W[sl] * WSCALE).transpose(2, 0, 1).reshape(
            KT2, 2, 128, PL * Q).transpose(0, 2, 1, 3)
        xw = np.empty((KT2, 128, 2, XC + WC), dtype=f8)
        xw[:, :, :, :XC] = xT
        xw[:, :, :, XC:] = Wt.astype(f8)
        bo = np.empty((1, WC + 128), dtype=bf)
        bo[0, :WC] = (b[sl].reshape(-1) * WSCALE).astype(bf)
        bo[0, WC:] = 1.0
        oh = (qarange[:, None, None] == part_idx[sl][None, :, :]
              ).reshape(128, PL * C).astype(f8)
        in_maps.append({"xw": xw, "bo": bo, "ident": ident, "oh": oh})
    return in_maps


def kernel(x, W, b, part_idx, _trace=False):
    from concourse.bass_utils import run_bass_kernel_spmd

    x = np.asarray(x, dtype=np.float32)
    W = np.asarray(W, dtype=np.float32)
    b = np.asarray(b, dtype=np.float32)
    part_idx = np.asarray(part_idx)

    nc = _build_nc()
    in_maps = _host_inputs(x, W, b, part_idx)
    res = run_bass_kernel_spmd(nc, in_maps, list(range(NCORES)),
                               trace=_trace)
    out = np.empty((B, P, C), dtype=np.float32)
    for i, r in enumerate(res.results):
        out[:, i * PL:(i + 1) * PL, :500] = np.asarray(r["out"],
                                                       dtype=np.float32)
        out[:, i * PL:(i + 1) * PL, 500:] = np.asarray(r["out2"],
                                                       dtype=np.float32)
    if _trace:
        return out, res
    return out


# revision 6
# speedup vs baseline: 1.0478x; 1.0478x over previous
"""Trainium2 Bass kernel for nn_CombinatorialClassifier.

Computation (reference):
    logits = einsum('bf,pqf->bpq', x, W) + b        # [B,P,Q]
    logp   = log_softmax(logits, axis=2)            # [B,P,Q]
    out    = take_along_axis(logp, part_idx, 2)     # [B,P,C]

Shapes: B=256, P=64, Q=128, C=1000, F=2048.  Expert-parallel over P
(8 partitionings per core), full x on every core, no collectives.

Design (~1.7x faster than the v1 one-hot/orientation-A kernel):
  - main matmul in orientation B: psum_lin[b, (p,q)] += x_k.T @ W_k
    with x stationary and fp8e4 DoubleRow (2 contraction rows per
    partition: half the HBM bytes, 2x PE rate; rel err 6.8e-3 vs the
    2e-2 gate).  W is scaled by 32 on the host; the 1/32 descale rides
    the ACT/DVE copies.  Mains stream k-major behind the input DMAs.
  - softmax: wide ACT exps + DVE tensor_reduce give sumexp in the
    [b, 1] orientation for free (no PE sumexp, no lse transposes); lse
    is folded per-partition into the og drain copies, so the gather
    operates on raw rescaled logits.
  - gather is a one-hot matmul; the one-hot is host-built fp8e4 (exact
    0/1, mixed bf16 x fp8 operands).  Rescaled logits are transposed
    to [q, b] via PE transposes (identity DMA'd once) into per-blk
    bf16 psum tiles, copied to SBUF by DVE.
  - the psum->SBUF og drain alternates per 500-col chunk between DVE
    (tensor_scalar subtract) and ACT (Identity + neg_lse bias) over 6
    rotating one-bank psum slots, so both engines drain concurrently
    and every gather matmul carries exactly ONE semaphore wait (this
    walrus build encodes at most one sync wait per instruction; the
    chunk-class/slot-parity assignment makes each slot WAR land on the
    same semaphore as the gather's data wait).
  - outputs are bf16 halves (host stitches + upcasts): c0 via SP-issued
    HWDGE DMAs (the input queue is provably drained, so the legalizer
    drops the queue-pred wait), c1 via Pool-issued SWDGE DMAs behind a
    Pool observer.
  - _install_drain_split post-processes the serialized BIR with a
    vector-clock pass that drops transitively-implied semaphore waits
    and splits the tail Drains, enforcing the single-wait encoding.
"""

import numpy as np

B, P, Q, C, F = 256, 64, 128, 1000, 2048
NCORES = 8
PL = P // NCORES          # partitionings per core
KT2 = 8                   # K tiles of 256 (128 partitions x DoubleRow 2)
XC = B                    # x columns in the xw stream
WC = PL * Q               # W columns in the xw stream
NBLK = B // 128           # b blocks
WSCALE = 32.0


def _build_nc():
    import concourse.bass as bass
    import concourse.tile as tile
    from concourse import mybir
    from contextlib import ExitStack

    DT = mybir.dt.float32
    BF = mybir.dt.bfloat16
    F8 = mybir.dt.float8e4
    ACT = mybir.ActivationFunctionType

    nc = bass.Bass()
    xw_d = nc.declare_dram_parameter("xw", [KT2, 128, 2, XC + WC], F8,
                                     isOutput=False)
    bo_d = nc.declare_dram_parameter("bo", [1, WC + 128], BF, isOutput=False)
    id_d = nc.declare_dram_parameter("ident", [128, 128], BF, isOutput=False)
    oh_d = nc.declare_dram_parameter("oh", [128, PL * C], F8, isOutput=False)
    out_d = nc.declare_dram_parameter("out", [B, PL, 500], BF, isOutput=True)
    out2_d = nc.declare_dram_parameter("out2", [B, PL, 500], BF,
                                       isOutput=True)

    with ExitStack() as ctx:
        tc = ctx.enter_context(tile.TileContext(nc))
        singles = ctx.enter_context(tc.tile_pool(name="singles", bufs=1))
        ps_t = ctx.enter_context(
            tc.tile_pool(name="ps_t", bufs=1, space=bass.MemorySpace.PSUM))
        lin_ctx = ExitStack()
        ps_lin = lin_ctx.enter_context(
            tc.tile_pool(name="ps_lin", bufs=1, space=bass.MemorySpace.PSUM))

        def fresh(shape, dtype, tag):
            return singles.tile(shape, dtype, tag=tag, name=tag)

        # ---- input DMAs, all on the SP HWDGE queue (ordered sems) ---
        bo_sb = fresh([1, WC + 128], BF, "bo")
        nc.sync.dma_start(out=bo_sb[:], in_=bo_d[:])
        id_sb = fresh([128, 128], BF, "ident")
        nc.sync.dma_start(out=id_sb[:], in_=id_d[:])
        xwk = []
        for k in range(KT2):
            t = fresh([128, 2, XC + WC], F8, f"xwk{k}")
            nc.sync.dma_start(out=t[:], in_=xw_d[k])
            xwk.append(t)
        oh_sb = fresh([128, PL * C], F8, "oh")
        nc.sync.dma_start(out=oh_sb[:], in_=oh_d[:])

        pst = {}
        for blk in range(NBLK):
            pst[blk] = ps_t.tile([128, PL, 128], BF, name=f"pst{blk}")

        # ---- PE: warmups, bias openers + DoubleRow mains -------------
        lin = {}
        for blk in range(NBLK):
            for h in range(2):
                t = ps_lin.tile([128, 4, 128], DT, name=f'lin{blk}_{h}')
                lin[(blk, h)] = t
        # dummy transpose consumes ident's DMA sem early
        nc.tensor.transpose(pst[0][:, 0, :], id_sb[:], id_sb[:])
        for blk in range(NBLK):
            for h in range(2):
                nc.tensor.matmul(
                    lin[(blk, h)][:, :, :],
                    bo_sb[0:1, WC:WC + 128],
                    bo_sb[0:1, h * 512:(h + 1) * 512],
                    start=True, stop=False)
        for blk in range(NBLK):
            for k in range(KT2):
                for h in range(2):
                    nc.tensor.matmul(
                        lin[(blk, h)][:, :, :],
                        xwk[k][:, :, blk * 128:(blk + 1) * 128],
                        xwk[k][:, :, XC + h * 512:XC + (h + 1) * 512],
                        start=False, stop=(k == KT2 - 1),
                        perf_mode=mybir.MatmulPerfMode.DoubleRow)

        # PE observer: absorb oh's DMA sem so gathers carry only their
        # drain-chain wait
        nc.tensor.ldweights(oh_sb[:, 0:1])

        # ---- softmax prologue per blk --------------------------------
        obs_junk = fresh([1, 16], DT, "obs_junk")
        warm_junk = fresh([1, 2], DT, "warm_junk")
        nc.scalar.activation(out=warm_junk[0:1, 0:1],
                             in_=warm_junk[0:1, 1:2],
                             func=ACT.Identity, scale=0.0, bias=0.0)

        linsc, lse, neg_lse, exps, sumexp = {}, {}, {}, {}, {}

        def mk_linsc(blk, eng):
            for h in range(2):
                t = fresh([128, 4, 128], BF, f"linsc{blk}_{h}")
                linsc[(blk, h)] = t
                if eng == 'act':
                    nc.scalar.activation(out=t[:, :, :],
                                         in_=lin[(blk, h)][:, :, :],
                                         func=ACT.Copy, scale=1.0 / WSCALE,
                                         bias=0.0)
                else:
                    nc.vector.tensor_scalar_mul(t[:, :, :],
                                                lin[(blk, h)][:, :, :],
                                                1.0 / WSCALE)

        def act_exp(blk):
            for h in range(2):
                e = fresh([128, 4, 128], BF, f"exp{blk}_{h}")
                exps[(blk, h)] = e
                nc.scalar.activation(out=e[:, :, :], in_=lin[(blk, h)][:, :, :],
                                     func=ACT.Exp, scale=1.0 / WSCALE)

        def dve_red(blk):
            sumexp[blk] = fresh([128, PL], DT, f"sumexp{blk}")
            for h in range(2):
                nc.vector.tensor_reduce(
                    out=sumexp[blk][:, h * 4:(h + 1) * 4],
                    in_=exps[(blk, h)][:, :, :],
                    axis=mybir.AxisListType.X, op=mybir.AluOpType.add)

        def act_post(blk):
            t = fresh([128, PL], DT, f"lse{blk}")
            lse[blk] = t
            nc.scalar.activation(out=t[:], in_=sumexp[blk][:], func=ACT.Ln)
            t2 = fresh([128, PL], DT, f"neglse{blk}")
            neg_lse[blk] = t2
            nc.scalar.activation(out=t2[:], in_=lse[blk][:],
                                 func=ACT.Identity, scale=-1.0)
            # ACT self-absorber for the neg_lse RAW
            aabs = fresh([1, 1], DT, f"aabs{blk}")
            nc.scalar.activation(out=aabs[:], in_=neg_lse[blk][0:1, 0:1],
                                 func=ACT.Copy, bias=0.0, scale=1.0)

        act_exp(0)
        act_exp(1)
        mk_linsc(0, 'act')
        mk_linsc(1, 'act')
        dve_red(0)
        dve_red(1)

        # ---- transposes + logpT copies (all copies on DVE) -----------
        logpT = {}
        for blk in range(NBLK):
            for p in range(PL):
                nc.tensor.transpose(pst[blk][:, p, :],
                                    linsc[(blk, p // 4)][:, p % 4, :],
                                    id_sb[:])
        for blk in range(NBLK):
            for p in range(PL):
                t = fresh([128, 128], BF, f"logpT{blk}_{p}")
                logpT[(blk, p)] = t
                nc.vector.tensor_copy(t[:], pst[blk][:, p, :])
        act_post(0)
        act_post(1)
        # DVE absorbers: pull ACT@neg_lse into DVE's clock before ogs
        dabs0 = fresh([128, 1], DT, "dabs0")
        nc.vector.tensor_copy(dabs0[:], neg_lse[0][:, 0:1])
        dabs1 = fresh([128, 1], DT, "dabs1")
        nc.vector.tensor_copy(dabs1[:], neg_lse[1][:, 0:1])

        # lin banks free; 6 one-bank chunk slots (pst banks stay live)
        lin_ctx.close()
        ps_out = ctx.enter_context(
            tc.tile_pool(name="ps_out", bufs=6, space=bass.MemorySpace.PSUM))

        # PE LDW observer: absorb ACT's lin-bank reads (exps + linscs)
        # so first-rotation gathers' bank-reuse WARs are covered
        nc.tensor.ldweights(linsc[(1, 1)][:, 0, 0:1])


        og_c0, og_c1 = {}, {}
        og_c0_last, og_c1_last = {}, {}

        def gather_p(blk, p):
            pair = p // 2
            if p % 2 == 0:
                og_c0[(blk, pair)] = fresh([128, 2, 500], BF,
                                           f"ogc0_{blk}_{pair}")
                og_c1[(blk, pair)] = fresh([128, 2, 500], BF,
                                           f"ogc1_{blk}_{pair}")
            for ci in range(2):
                po = ps_out.tile([128, 512], DT, name='po')
                nc.tensor.matmul(
                    po[:, 0:500], logpT[(blk, p)][:],
                    oh_sb[:, p * C + ci * 500:p * C + ci * 500 + 500],
                    start=True, stop=True)
                if ci == 0:
                    # c0 chunks drain on DVE (fused lse subtract)
                    og_c0_last[(blk, pair)] = nc.vector.tensor_scalar(
                        out=og_c0[(blk, pair)][:, p % 2, :],
                        in0=po[:, 0:500],
                        scalar1=lse[blk][:, p:p + 1], scalar2=None,
                        op0=mybir.AluOpType.subtract)
                else:
                    # c1 chunks drain on ACT
                    og_c1_last[(blk, pair)] = nc.scalar.activation(
                        out=og_c1[(blk, pair)][:, p % 2, :], in_=po[:, 0:500],
                        func=ACT.Identity, scale=1.0,
                        bias=neg_lse[blk][:, p:p + 1])
            if p % 2 == 1:
                bsl = slice(blk * 128, (blk + 1) * 128)
                # c0 out: SP-issued (input queue provably drained, so the
                # legalizer drops the queue-pred wait; single DVE sem)
                dma0 = nc.sync.dma_start(
                    out=out_d[bsl, pair * 2:pair * 2 + 2, :],
                    in_=og_c0[(blk, pair)][:])
                tile.add_dep_helper(dma0.ins, og_c0_last[(blk, pair)].ins,
                                    sync=False, reason="dma after og c0")
                # c1 out: Pool-issued SWDGE behind a Pool observer; the
                # final pair goes out per-p so the last transfer is half
                # the size and p6's data leaves as soon as it drains
                if pair == 3:
                    for pp in range(2):
                        obs = nc.gpsimd.tensor_copy(
                            obs_junk[0:1, 8 + blk * 2 + pp:9 + blk * 2 + pp],
                            og_c1[(blk, pair)][0:1, pp, 499:500])
                        dma1 = nc.gpsimd.dma_start(
                            out=out2_d[bsl, pair * 2 + pp, :],
                            in_=og_c1[(blk, pair)][:, pp, :])
                        tile.add_dep_helper(dma1.ins, obs.ins, sync=False,
                                            reason="dma after pool obs")
                else:
                    obs = nc.gpsimd.tensor_copy(
                        obs_junk[0:1, blk * 4 + pair:blk * 4 + pair + 1],
                        og_c1[(blk, pair)][0:1, 1, 499:500])
                    dma1 = nc.gpsimd.dma_start(
                        out=out2_d[bsl, pair * 2:pair * 2 + 2, :],
                        in_=og_c1[(blk, pair)][:])
                    tile.add_dep_helper(dma1.ins, obs.ins, sync=False,
                                        reason="dma after pool obs")

        # ---- per-p interleaved gathers: both drain chains run hot ----
        for p in range(PL):
            gather_p(0, p)
            gather_p(1, p)

    _install_drain_split(nc)
    return nc


def _install_drain_split(nc, chunk=1):
    """Legalize sync for this walrus build (at most ONE sync wait per
    instruction):

    1. Vector-clock pass: compute, for every instruction, the set of
       instructions provably COMPLETED before it dispatches — via its
       own sem waits (a wait S>=v proves every update contributing to
       values 1..v completed, and transitively everything those
       instructions' dispatch-clocks contain) plus same-engine dispatch
       order (an engine dispatches in program order, so anything done
       before a predecessor's dispatch is done before ours).  Any
       emitted wait already implied by the rest is dropped.
    2. Remaining multi-wait Drains are split into single-wait chains.
    """
    import copy
    import json

    orig = nc.to_json_bytes

    def patched():
        m = json.loads(orig())
        for fn in m["functions"]:
            insts = []
            for bb in fn["blocks"]:
                insts.extend(bb["instructions"])
            n = len(insts)
            # sem name -> list of (cum_value, idx) in completion order
            updates = {}
            cum = {}
            for i, inst in enumerate(insts):
                si = inst.get("sync_info") or {}
                for u in (si.get("on_update") or []):
                    s = u["ant_name"]
                    cum[s] = cum.get(s, 0) + u.get("update_value", 1)
                    updates.setdefault(s, []).append((cum[s], i))

            def targets(s, v):
                """instruction idxs whose updates are needed for sem s
                to reach v"""
                return [i for (c, i) in updates.get(s, []) if c <= v]

            eng_pred = {}
            last = {}
            for i, inst in enumerate(insts):
                e = inst.get("engine", "?")
                eng_pred[i] = last.get(e)
                last[e] = i

            done = [set() for _ in range(n)]
            for _ in range(64):
                changed = False
                for i, inst in enumerate(insts):
                    d = set()
                    if eng_pred[i] is not None:
                        p = eng_pred[i]
                        d |= done[p]
                    si = inst.get("sync_info") or {}
                    for w in (si.get("on_wait") or []):
                        for j in targets(w["ant_name"], w["wait_value"]):
                            d.add(j)
                            d |= done[j]
                    if d != done[i]:
                        done[i] = d
                        changed = True
                if not changed:
                    break

            # drop implied waits on multi-wait instructions
            for i, inst in enumerate(insts):
                si = inst.get("sync_info") or {}
                waits = si.get("on_wait") or []
                if len(waits) <= 1:
                    continue
                base = set()
                if eng_pred[i] is not None:
                    base |= done[eng_pred[i]]
                keep = list(waits)
                for w in list(keep):
                    others = set(base)
                    for w2 in keep:
                        if w2 is w:
                            continue
                        for j in targets(w2["ant_name"], w2["wait_value"]):
                            others.add(j)
                            others |= done[j]
                    if all(j in others
                           for j in targets(w["ant_name"], w["wait_value"])):
                        keep.remove(w)
                        if len(keep) <= 1:
                            break
                si["on_wait"] = keep

        # split any remaining multi-wait Drains
        for fn in m["functions"]:
            for bb in fn["blocks"]:
                out = []
                for inst in bb["instructions"]:
                    si = inst.get("sync_info")
                    if (si and si.get("on_wait")
                            and len(si["on_wait"]) > chunk):
                        if inst.get("opcode") != "Drain":
                            raise RuntimeError(
                                f"multi-wait survives legalization: "
                                f"{inst.get('opcode')} {inst.get('name')} "
                                f"{si['on_wait']}")
                        waits = si["on_wait"]
                        head, keep = waits[:-chunk], waits[-chunk:]
                        for j in range(0, len(head), chunk):
                            clone = copy.deepcopy(inst)
                            clone["name"] = f"{inst['name']}-ds{j}"
                            clone["sync_info"] = {
                                "on_wait": head[j:j + chunk],
                                "on_update": [],
                            }
                            out.append(clone)
                        si["on_wait"] = keep
                    out.append(inst)
                bb["instructions"] = out
        return json.dumps(m).encode()

    nc.to_json_bytes = patched


def _host_inputs(x, W, b, part_idx):
    import ml_dtypes
    f8 = ml_dtypes.float8_e4m3
    bf = ml_dtypes.bfloat16

    # x: [B, F] -> [KT2, 128, 2, B] with f = 256*k2 + 128*j + r
    xT = np.ascontiguousarray(
        x.T.reshape(KT2, 2, 128, B).transpose(0, 2, 1, 3)).astype(f8)
    ident = np.eye(128, dtype=np.float32).astype(bf)
    qarange = np.arange(Q, dtype=np.int64)

    in_maps = []
    for i in range(NCORES):
        sl = slice(i * PL, (i + 1) * PL)
        Wt = (W[sl] * WSCALE).transpose(2, 0, 1).reshape(
            KT2, 2, 128, PL * Q).transpose(0, 2, 1, 3)
        xw = np.empty((KT2, 128, 2, XC + WC), dtype=f8)
        xw[:, :, :, :XC] = xT
        xw[:, :, :, XC:] = Wt.astype(f8)
        bo = np.empty((1, WC + 128), dtype=bf)
        bo[0, :WC] = (b[sl].reshape(-1) * WSCALE).astype(bf)
        bo[0, WC:] = 1.0
        oh = (qarange[:, None, None] == part_idx[sl][None, :, :]
              ).reshape(128, PL * C).astype(f8)
        in_maps.append({"xw": xw, "bo": bo, "ident": ident, "oh": oh})
    return in_maps


def kernel(x, W, b, part_idx, _trace=False):
    from concourse.bass_utils import run_bass_kernel_spmd

    x = np.asarray(x, dtype=np.float32)
    W = np.asarray(W, dtype=np.float32)
    b = np.asarray(b, dtype=np.float32)
    part_idx = np.asarray(part_idx)

    nc = _build_nc()
    in_maps = _host_inputs(x, W, b, part_idx)
    res = run_bass_kernel_spmd(nc, in_maps, list(range(NCORES)),
                               trace=_trace)
    out = np.empty((B, P, C), dtype=np.float32)
    for i, r in enumerate(res.results):
        out[:, i * PL:(i + 1) * PL, :500] = np.asarray(r["out"],
                                                       dtype=np.float32)
        out[:, i * PL:(i + 1) * PL, 500:] = np.asarray(r["out2"],
                                                       dtype=np.float32)
    if _trace:
        return out, res
    return out
